# revision 59
# baseline (speedup 1.0000x reference)
"""Trainium2 Bass kernel for nn_AttentionModule (sparse_attention).

Strategy (8 NeuronCores, no collectives):
  The 16 heads x 4 batches decompose into 32 (batch, head-pair) units.
  Each unit owns 2 heads of one batch: its 128 attn-dim columns of
  Wq/Wk/Wv (projection + LARoPE), QK^T, softmax, PV, and its 128-row
  slice of the out-projection, producing a PARTIAL y[b] that the host
  sums (plus the bo bias term).  Units are trimmed to the per-batch
  valid lengths (Q_b = query prefix rounded up to 128, K_b = key prefix
  rounded up to 64 - dropped tails are fully masked so they contribute
  exact zeros) and packed onto the 8 cores by a greedy balance on
  predicted PE rows.  Each distinct per-core shape signature compiles
  its own program (cached); all matmuls bf16 with fp32 PSUM.

  Schedule notes (from traces): PE matmul cost on trn2 is free-dim rows
  only (K=64 / M=65 matmuls are NOT penalized), but any PE gap resets a
  3us half-clock P-state window, so everything is woven to keep the PE
  streaming: kproj(g) | QK(g-1) | PV(g-2) software pipeline, segment
  tails woven with the next segment's Q/V phase, softmax-denominator
  reciprocal via reciprocal_approx_fast off a single-ACT-copy PSUM
  eviction, and host-packed single-DMA input tensors (per-DMA issue
  costs ~0.7us serialized on the Sync queue).

  Measured (8 cores concurrent, per-core NTFF): max 158.3us/core
  (all cores within 155.2-158.3us), rel err 3.77e-3 vs the fp32
  reference (baseline was 297us).  Norm-chain PSUM evictions run on
  the DVE - the ACT engine is exp-saturated and paces the attention
  phase through the 2-deep QK PSUM pool.
"""

import contextlib
import math
import os
import sys

import numpy as np


def _ensure_paths():
    for p in ("/opt/trn_rl_repo", "/root/.axon_site/_ro/trn_rl_repo"):
        if os.path.isdir(p) and p not in sys.path:
            sys.path.insert(0, p)


try:
    import concourse.bass as bass  # noqa: F401
except ImportError:
    _ensure_paths()

import ml_dtypes
import concourse.bass as bass  # noqa: F401
import concourse.tile as tile
from concourse import bacc, bass2jax, mybir

B = 4
T = 1024
L = 1024
DM = 1024
AD = 1024
H = 16
N_CORES = 8
SCALE = 1.0 / math.sqrt(AD)
ROPE_GAMMA = 10.0
ROTARY_BASE = 10000.0
MASK_BIAS = -30000.0

MDT = mybir.dt.bfloat16
NP_MDT = ml_dtypes.bfloat16
F8 = mybir.dt.float8e4
NP_F8 = ml_dtypes.float8_e4m3fn
F32 = mybir.dt.float32
AL = mybir.AluOpType
AF = mybir.ActivationFunctionType
DRPM = mybir.MatmulPerfMode.DoubleRow


def _fp8x2_mm(nc, out_ps, lhs_hi, lhs_lo, rhs_hi, rhs_lo, extra_tail=None):
    """Emit a 3-pass fp8x2 DoubleRow matmul chain into out_ps.

    Each operand is a callable i -> AP giving the [128, 2, n] slice for
    DR k-tile pair i (i in 0..3, covering dm chunks 2i, 2i+1).  Passes:
    hi*hi, hi(lhs)*lo(rhs), lo(lhs)*hi(rhs).  extra_tail: emitted with
    stop on the last matmul instead (e.g. bias row add).
    """
    chains = [(lhs_hi, rhs_hi), (lhs_hi, rhs_lo), (lhs_lo, rhs_hi)]
    n = 0
    for lhs_f, rhs_f in chains:
        for i in range(4):
            first = n == 0
            last = (n == 11) and extra_tail is None
            nc.tensor.matmul(out_ps, lhs_f(i), rhs_f(i),
                             start=first, stop=last, perf_mode=DRPM)
            n += 1
    if extra_tail is not None:
        extra_tail()

ATTN_BUFS = 16
QK_SCALE = 16.0  # fp8 qT/kT pre-scale, folded into trig tables


def _chunks(n, c=512):
    out, o = [], 0
    while o < n:
        w = min(c, n - o)
        out.append((o, w))
        o += w
    return out


def _ltiles(k):
    return _chunks(k, 128)


# ----------------------------------------------------------------------
# Program builder: one program per core-shape signature.
# sig = tuple of (Q, K, NP) per segment.
# ----------------------------------------------------------------------

def build_program(sig, dbg=False):
    nc = bacc.Bacc("TRN2", target_bir_lowering=False, debug=False)

    def din(name, shape, dt):
        return nc.dram_tensor(name, shape, dt, kind="ExternalInput").ap()

    onesr = din("onesr", [1, 128], MDT)
    perm = din("perm", [128, 128], MDT)

    segio = []
    for si, (Q, K, NP) in enumerate(sig):
        nlt = len(_ltiles(K))
        segio.append(dict(
            xs=din(f"xs{si}", [128, 8 * Q], MDT),
            ctx=din(f"ctx{si}", [128, 8 * K], MDT),
            wq=din(f"wq{si}", [128, 1024 * NP], MDT),
            wk=din(f"wk{si}", [128, 1024 * NP], MDT),
            wv=din(f"wv{si}", [128, 8 * 128 * NP], MDT),
            wo=din(f"wo{si}", [128, 1024 * NP], MDT),
            trig=din(f"tg{si}", [128, 2 * Q + 2 * K], MDT),
            cst=din(f"cst{si}", [128, 2 * NP + nlt], F32),
            bvr=din(f"bvr{si}", [1, 128 * NP], MDT),
            y=nc.dram_tensor(f"y{si}", [128, 8 * Q], MDT,
                             kind="ExternalOutput").ap(),
        ))

    with tile.TileContext(nc) as tc, contextlib.ExitStack() as ctx:
        sb = ctx.enter_context(tc.tile_pool(name="sb", bufs=1))
        ps = ctx.enter_context(tc.tile_pool(name="ps", bufs=2, space="PSUM"))

        C_ones = sb.tile([1, 128], MDT, tag="ones", bufs=1, name="ones")
        C_perm = sb.tile([128, 128], MDT, tag="perm", bufs=1, name="permt")

        # ---- all input DMAs up front, ordered by first use --------------
        st = []  # per-seg sbuf input tiles
        for si, (Q, K, NP) in enumerate(sig):
            io = segio[si]
            d = {}

            def ld(key, ap, sx=si):
                t = sb.tile(list(ap.shape), ap.dtype, tag=f"in{sx}_{key}",
                            bufs=1, name=f"in{sx}_{key}")
                nc.sync.dma_start(t[:], ap)
                return t

            d["cst"] = ld("cst", io["cst"])
            d["xs"] = ld("xs", io["xs"])
            d["wq"] = ld("wq", io["wq"])
            if si == 0:
                nc.sync.dma_start(C_perm[:], perm)
                nc.sync.dma_start(C_ones[:], onesr)
            d["trig"] = ld("trig", io["trig"])
            d["ctx"] = ld("ctx", io["ctx"])
            d["wk"] = ld("wk", io["wk"])
            d["wv"] = ld("wv", io["wv"])
            d["bvr"] = ld("bvr", io["bvr"])
            d["wo"] = ld("wo", io["wo"])
            st.append(d)

        # ---- per-segment emission helpers -------------------------------
        def seg_chunks(si):
            Q, K, NP = sig[si]
            io, d = segio[si], st[si]
            qcs = _chunks(Q)
            kcs = _chunks(K)
            lts = _ltiles(K)
            nlt = len(lts)
            nh = 2 * NP

            qT = [None] * NP
            kT = [None] * NP
            vP = [None] * nlt
            osb = [None] * NP
            attn = {}

            # -- Q phase ---------------------------------------------------
            pend_q = []

            def flush_q():
                a, q0, qw, wsb, asb = pend_q.pop(0)
                pw = ps.tile([128, qw], F32, tag="pp", bufs=4,
                             name=f"s{si}qpw{a}_{q0}")
                nc.tensor.matmul(pw[:], C_perm[:], wsb[:], start=True,
                                 stop=True)
                nc.vector.tensor_add(qT[a][:, q0:q0 + qw], pw[:], asb[:])

            def q_unit(a, j):
                q0, qw = qcs[j]
                if qT[a] is None:
                    qT[a] = sb.tile([128, Q], MDT, tag=f"qT{si}", bufs=NP,
                                    name=f"s{si}qT{a}")
                q_ps = ps.tile([128, qw], F32, tag="pp", bufs=4,
                               name=f"s{si}qps{a}_{j}")
                for dd in range(8):
                    nc.tensor.matmul(
                        q_ps[:],
                        d["wq"][:, a * 1024 + dd * 128:
                                a * 1024 + (dd + 1) * 128],
                        d["xs"][:, dd * Q + q0:dd * Q + q0 + qw],
                        start=(dd == 0), stop=(dd == 7))
                wsb = sb.tile([128, qw], MDT, tag="ropeW", bufs=3,
                              name=f"s{si}qw{a}_{j}")
                nc.vector.scalar_tensor_tensor(
                    wsb[:], q_ps[:], d["cst"][:, a:a + 1],
                    d["trig"][:, Q + q0:Q + q0 + qw],
                    op0=AL.add, op1=AL.mult)
                asb = sb.tile([128, qw], MDT, tag="ropeA", bufs=3,
                              name=f"s{si}qa{a}_{j}")
                nc.vector.scalar_tensor_tensor(
                    asb[:], q_ps[:], d["cst"][:, a:a + 1],
                    d["trig"][:, q0:q0 + qw],
                    op0=AL.add, op1=AL.mult)
                pend_q.append((a, q0, qw, wsb, asb))
                if len(pend_q) > 1:
                    flush_q()

            # -- V phase ---------------------------------------------------
            vhs = [(0, nh)] if 65 * nh <= 512 else [(0, nh // 2),
                                                    (nh // 2, nh - nh // 2)]

            def v_unit(lt):
                l0, lsz = lts[lt]
                vt = sb.tile([128, 65 * nh], MDT, tag=f"vP{si}", bufs=nlt,
                             name=f"s{si}vP{lt}")
                vP[lt] = vt
                for hh0, hn in vhs:
                    v_ps = ps.tile([128, 64 * hn], F32, tag="pp", bufs=4,
                                   name=f"s{si}vps{lt}_{hh0}")
                    for dd in range(8):
                        nc.tensor.matmul(
                            v_ps[0:lsz, :],
                            d["ctx"][:, dd * K + l0:dd * K + l0 + lsz],
                            d["wv"][:, dd * 128 * NP + hh0 * 64:
                                    dd * 128 * NP + (hh0 + hn) * 64],
                            start=(dd == 0), stop=False)
                    nc.tensor.matmul(
                        v_ps[0:lsz, :], C_ones[0:1, 0:lsz],
                        d["bvr"][0:1, hh0 * 64:(hh0 + hn) * 64],
                        start=False, stop=True)
                    out_ap = vt[0:lsz, hh0 * 65:(hh0 + hn) * 65].rearrange(
                        "p (h e) -> p h e", e=65)[:, :, 0:64]
                    in_ap = v_ps[0:lsz, :].rearrange("p (h d) -> p h d", d=64)
                    nc.scalar.copy(out_ap, in_ap)
                ones_ap = vt[0:lsz, :].rearrange(
                    "p (h e) -> p h e", e=65)[:, :, 64:65]
                nc.gpsimd.memset(ones_ap, 1.0)

            # -- K phase ---------------------------------------------------
            pend_k = []

            def flush_k():
                g, k0, kw, wsb, asb = pend_k.pop(0)
                pw = ps.tile([128, kw], F32, tag="pp", bufs=4,
                             name=f"s{si}kpw{g}_{k0}")
                nc.tensor.matmul(pw[:], C_perm[:], wsb[:], start=True,
                                 stop=True)
                nc.vector.tensor_add(kT[g][:, k0:k0 + kw], pw[:], asb[:])

            def k_unit(g, j):
                k0, kw = kcs[j]
                if kT[g] is None:
                    kT[g] = sb.tile([128, K], MDT, tag=f"kT{si}", bufs=NP,
                                    name=f"s{si}kT{g}")
                k_ps = ps.tile([128, kw], F32, tag="pp", bufs=4,
                               name=f"s{si}kps{g}_{j}")
                for dd in range(8):
                    nc.tensor.matmul(
                        k_ps[:],
                        d["wk"][:, g * 1024 + dd * 128:
                                g * 1024 + (dd + 1) * 128],
                        d["ctx"][:, dd * K + k0:dd * K + k0 + kw],
                        start=(dd == 0), stop=(dd == 7))
                wsb = sb.tile([128, kw], MDT, tag="ropeW", bufs=3,
                              name=f"s{si}kw{g}_{j}")
                nc.vector.scalar_tensor_tensor(
                    wsb[:], k_ps[:], d["cst"][:, NP + g:NP + g + 1],
                    d["trig"][:, 2 * Q + K + k0:2 * Q + K + k0 + kw],
                    op0=AL.add, op1=AL.mult)
                asb = sb.tile([128, kw], MDT, tag="ropeA", bufs=3,
                              name=f"s{si}ka{g}_{j}")
                nc.vector.scalar_tensor_tensor(
                    asb[:], k_ps[:], d["cst"][:, NP + g:NP + g + 1],
                    d["trig"][:, 2 * Q + k0:2 * Q + k0 + kw],
                    op0=AL.add, op1=AL.mult)
                pend_k.append((g, k0, kw, wsb, asb))
                if len(pend_k) > 1:
                    flush_k()

            # -- QK + exp --------------------------------------------------
            def qk_unit(g, lt, j):
                # head h2's slice sits at column offset h2*512 — matmul
                # PSUM writes must start on a 2KB bank boundary (offset
                # qw<512 faults the device).
                l0, lsz = lts[lt]
                q0, qw = qcs[j]
                qk = ps.tile([128, 1024], F32, tag="qk", bufs=2,
                             name=f"s{si}qk{g}_{lt}_{j}")
                for h2 in range(2):
                    nc.tensor.matmul(
                        qk[0:lsz, h2 * 512:h2 * 512 + qw],
                        kT[g][h2 * 64:(h2 + 1) * 64, l0:l0 + lsz],
                        qT[g][h2 * 64:(h2 + 1) * 64, q0:q0 + qw],
                        start=True, stop=True)
                at = sb.tile([128, 1024], MDT, tag="attn", bufs=ATTN_BUFS,
                             name=f"s{si}at{g}_{lt}_{j}")
                bias = d["cst"][0:lsz, 2 * NP + lt:2 * NP + lt + 1]
                nc.scalar.activation(
                    at[0:lsz, 0:512 + qw], qk[0:lsz, 0:512 + qw], AF.Exp,
                    bias=bias, scale=SCALE)
                attn[(g, lt, j)] = at

            # -- PV + normalize -------------------------------------------
            def pv_piece(g, h2, j, ltr, o_ps):
                q0, qw = qcs[j]
                hh = 2 * g + h2
                for lt in ltr:
                    l0, lsz = lts[lt]
                    nc.tensor.matmul(
                        o_ps[0:65, 0:qw],
                        vP[lt][0:lsz, hh * 65:(hh + 1) * 65],
                        attn[(g, lt, j)][0:lsz, h2 * 512:h2 * 512 + qw],
                        start=(lt == 0), stop=(lt == nlt - 1))

            def norm(g, h2, j, o_ps):
                q0, qw = qcs[j]
                dn = sb.tile([1, qw], F32, tag="dn", bufs=3,
                             name=f"s{si}dn{g}_{h2}_{j}")
                nc.vector.tensor_copy(dn[:], o_ps[64:65, 0:qw])
                rc = sb.tile([1, qw], F32, tag="rc", bufs=3,
                             name=f"s{si}rc{g}_{h2}_{j}")
                nc.vector.reciprocal_approx_fast(rc[:], dn[:])
                bc = sb.tile([64, qw], F32, tag="bc", bufs=3,
                             name=f"s{si}bc{g}_{h2}_{j}")
                nc.gpsimd.partition_broadcast(bc[:], rc[:], channels=64)
                nc.vector.tensor_mul(
                    osb[g][h2 * 64:(h2 + 1) * 64, q0:q0 + qw],
                    o_ps[0:64, 0:qw], bc[:])

            def pv_chunks(g, j):
                out = []
                if osb[g] is None:
                    osb[g] = sb.tile([128, Q], MDT, tag=f"osb{si}", bufs=NP,
                                     name=f"s{si}osb{g}")
                half = max(1, nlt // 2)
                for h2 in range(2):
                    box = {}

                    def p1(g=g, h2=h2, j=j, b=box):
                        b["o"] = ps.tile(
                            [65, qcs[j][1]], F32, tag="pp", bufs=4,
                            name=f"s{si}o{g}_{h2}_{j}")
                        pv_piece(g, h2, j, range(0, half), b["o"])

                    def p2(g=g, h2=h2, j=j, b=box):
                        pv_piece(g, h2, j, range(half, nlt), b["o"])
                        norm(g, h2, j, b["o"])

                    out.extend([p1, p2])
                return out

            # -- out projection -------------------------------------------
            yts = {}
            last_seg = si == len(sig) - 1

            def o_unit(dd, j):
                q0, qw = qcs[j]
                y_ps = ps.tile([128, qw], F32, tag="pp", bufs=4,
                               name=f"s{si}yps{dd}_{j}")
                for a in range(NP):
                    nc.tensor.matmul(
                        y_ps[:],
                        d["wo"][:, a * 1024 + dd * 128:
                                a * 1024 + (dd + 1) * 128],
                        osb[a][:, q0:q0 + qw],
                        start=(a == 0), stop=(a == NP - 1))
                if j not in yts:
                    yts[j] = sb.tile([128, 8 * qw], MDT, tag="yt", bufs=2,
                                     name=f"s{si}yt{j}")
                yt = yts[j]
                if dd % 2:
                    nc.vector.tensor_copy(yt[:, dd * qw:(dd + 1) * qw],
                                          y_ps[:])
                else:
                    nc.scalar.copy(yt[:, dd * qw:(dd + 1) * qw], y_ps[:])
                def dma_y(d0, dn, j=j, q0=q0, qw=qw, yt=yt):
                    nc.sync.dma_start(
                        segio[si]["y"].rearrange(
                            "p (d q) -> p d q", d=8)[:, d0:dn, q0:q0 + qw],
                        yt[:].rearrange("p (d q) -> p d q", d=8)[:, d0:dn])

                if last_seg and dd == 3:
                    dma_y(0, 4)
                elif dd == 7:
                    dma_y(4, 8) if last_seg else dma_y(0, 8)

            # ---- assemble chunk lists -----------------------------------
            # Pipeline items: s = (pair g, q-chunk j).  Step s runs
            # QK(item s-1) woven with PV+norm(item s-2) and the kproj
            # pieces of gang s//nqc + 1 (one gang ahead).  attn pool is
            # sized 2*nlt so item s's exp reuses item s-2's slots, whose
            # PV consumers were fully emitted at step s-1.
            nqc = len(qcs)
            items = [(g, j) for g in range(NP) for j in range(nqc)]

            heads = []
            for j in range(nqc):
                for a in range(NP):
                    heads.append(lambda a=a, j=j: q_unit(a, j))
            heads.append(lambda: [flush_q() for _ in range(len(pend_q))])
            for lt in range(nlt):
                heads.append(lambda lt=lt: v_unit(lt))
            for j in range(len(kcs)):
                heads.append(lambda j=j: k_unit(0, j))
            heads.append(lambda: [flush_k() for _ in range(len(pend_k))])

            def kp_pieces(s):
                g = s // nqc + 1
                if g >= NP:
                    return []
                out = []
                if nqc == 1 or s % nqc == 0:
                    out.append(lambda g=g: k_unit(g, 0))
                if nqc == 1 or s % nqc == 1:
                    if len(kcs) > 1:
                        out.append(lambda g=g: k_unit(g, 1))
                    out.append(
                        lambda: [flush_k() for _ in range(len(pend_k))])
                return out

            def gang_loop():
                for s in range(len(items)):
                    chunks = list(kp_pieces(s))
                    if s >= 2:
                        chunks.extend(pv_chunks(*items[s - 2]))
                    qks = []
                    if s >= 1:
                        g, j = items[s - 1]
                        for lt in range(nlt):
                            qks.append(
                                lambda g=g, lt=lt, j=j: qk_unit(g, lt, j))
                    for x in range(max(len(qks), len(chunks))):
                        if x < len(qks):
                            qks[x]()
                        if x < len(chunks):
                            chunks[x]()
                # one more step: QK of the last item
                g, j = items[-1]
                chunks = (list(pv_chunks(*items[-2]))
                          if len(items) >= 2 else [])
                qks = [lambda lt=lt: qk_unit(g, lt, j) for lt in range(nlt)]
                for x in range(max(len(qks), len(chunks))):
                    if x < len(qks):
                        qks[x]()
                    if x < len(chunks):
                        chunks[x]()

            def make_tail():
                out = list(pv_chunks(*items[-1]))
                for j in range(nqc):
                    for dd in range(8):
                        out.append(lambda dd=dd, j=j: o_unit(dd, j))
                return out

            return heads, gang_loop, make_tail, dict(
                qT=qT, kT=kT, vP=vP, osb=osb)

        # ---- top-level weave across segments ----------------------------
        prev_tail = []
        dbg_state = []
        for si in range(len(sig)):
            heads, gang_loop, make_tail, state = seg_chunks(si)
            dbg_state.append(state)
            n = max(len(prev_tail), len(heads))
            for x in range(n):
                if x < len(prev_tail):
                    prev_tail[x]()
                if x < len(heads):
                    heads[x]()
            gang_loop()
            prev_tail = make_tail()
        for c in prev_tail:
            c()
        if dbg:
            for si, state in enumerate(dbg_state):
                for nm, tiles in state.items():
                    for a, t in enumerate(tiles):
                        if t is None:
                            continue
                        shape = [t.shape[0], t.shape[1]]
                        ap = nc.dram_tensor(
                            f"dbg_{nm}{si}_{a}", shape, t.dtype,
                            kind="ExternalOutput").ap()
                        nc.sync.dma_start(ap, t[:])

    nc.compile()
    return nc


# ----------------------------------------------------------------------
# Planning: trim lengths from masks, balance 32 pair-units over 8 cores.
# ----------------------------------------------------------------------

def _pair_rows(Q, K):
    nlt = len(_ltiles(K))
    return 17 * Q + 9 * K + 4 * nlt * Q + 1170 * nlt


def _parts(n, k):
    out = []

    def rec(rem, k, mx, cur):
        if k == 1:
            if rem <= mx:
                out.append(cur + [rem])
            return
        for v in range(min(rem - k + 1, mx), 0, -1):
            rec(rem - v, k - 1, v, cur + [v])

    rec(n, k, n, [])
    return out


_PLAN_CACHE = {}


def _plan(QKs):
    """QKs: [(Q_b, K_b)] per batch. Returns per-core list of
    (batch, npairs); pairs of a batch are interchangeable.

    Exhaustive search over per-batch group-count partitions (each
    batch's 8 pairs split into 2-4 groups), LPT-assigned to the 8 cores
    under a <=2-batches-per-core SBUF constraint, minimizing max load.
    """
    import itertools

    key = tuple(QKs)
    if key in _PLAN_CACHE:
        return _PLAN_CACHE[key]
    u = [_pair_rows(q, k) for q, k in QKs]
    opts = []
    for b in range(B):
        o = []
        for k in (2, 3, 4):
            o.extend(_parts(8, k))
        opts.append(o)
    best = None
    for combo in itertools.product(*opts):
        groups = []
        for b, p in enumerate(combo):
            for sz in p:
                groups.append((sz * u[b], b, sz))
        if len(groups) > 2 * N_CORES:
            continue
        groups.sort(reverse=True)
        loads = [0.0] * N_CORES
        bats = [dict() for _ in range(N_CORES)]
        ok = True
        for w, b, sz in groups:
            cands = [c for c in range(N_CORES)
                     if b in bats[c] or len(bats[c]) < 2]
            if not cands:
                ok = False
                break
            c = min(cands, key=lambda c: loads[c])
            loads[c] += w
            bats[c][b] = bats[c].get(b, 0) + sz
        if not ok:
            continue
        mx = max(loads)
        if best is None or mx < best[0]:
            best = (mx, [sorted(d.items(), key=lambda t: -t[1] * u[t[0]])
                         for d in bats])
    if best is None:  # fallback: uniform half-head split
        plan = [[(c // 2, 4)] for c in range(N_CORES)]
    else:
        plan = best[1]
    _PLAN_CACHE[key] = plan
    return plan


# ----------------------------------------------------------------------
# Host prep / dispatch / assembly
# ----------------------------------------------------------------------

_PROG_CACHE = {}
_JIT_CACHE = {}

LAST_CORE_NCS = []  # nc per core, dispatch order (for test.py profiling)


def _get_program(sig):
    if sig not in _PROG_CACHE:
        _PROG_CACHE[sig] = build_program(sig)
    return _PROG_CACHE[sig]


def _jit_for(nc):
    import jax

    key = id(nc)
    if key in _JIT_CACHE:
        return _JIT_CACHE[key]
    partition_name = (
        nc.partition_id_tensor.name if nc.partition_id_tensor else None)
    in_names, out_names, out_avals, zero_outs = [], [], [], []
    for alloc in nc.m.functions[0].allocations:
        if not isinstance(alloc, mybir.MemoryLocationSet):
            continue
        name = alloc.memorylocations[0].name
        if alloc.kind == "ExternalInput":
            if name != partition_name:
                in_names.append(name)
        elif alloc.kind == "ExternalOutput":
            shape = tuple(alloc.tensor_shape)
            dtype = mybir.dt.np(alloc.dtype)
            out_names.append(name)
            out_avals.append(jax.core.ShapedArray(shape, dtype))
            zero_outs.append(np.zeros(shape, dtype))
    n_params = len(in_names)
    in_names_full = list(in_names) + list(out_names)
    if partition_name is not None:
        in_names_full.append(partition_name)
    in_names_full = tuple(in_names_full)

    def _body(*args):
        operands = list(args)
        if partition_name is not None:
            operands.append(bass2jax.partition_id_tensor())
        outs = bass2jax._bass_exec_p.bind(
            *operands, out_avals=tuple(out_avals), in_names=in_names_full,
            out_names=tuple(out_names), lowering_input_output_aliases=(),
            sim_require_finite=True, sim_require_nnan=True, nc=nc)
        return tuple(outs)

    donate_idx = tuple(range(n_params, n_params + len(out_names)))
    jfn = jax.jit(_body, donate_argnums=donate_idx, keep_unused=True)
    _JIT_CACHE[key] = (jfn, in_names, out_names, zero_outs)
    return _JIT_CACHE[key]


def _trim_lens(x_mask, context_mask):
    QKs = []
    for b in range(B):
        qnz = np.flatnonzero(np.asarray(x_mask[b, 0]))
        knz = np.flatnonzero(np.asarray(context_mask[b, 0]))
        qn = int(qnz.max()) + 1 if qnz.size else 1
        kn = int(knz.max()) + 1 if knz.size else 1
        QKs.append((min(T, -(-qn // 128) * 128), min(L, -(-kn // 64) * 64)))
    return QKs


def _pack8(arr, w):
    """[1024, X] -> [128, 8*w] blocks (d-major), X >= w columns used."""
    a = arr[:, :w] if arr.shape[1] != w else arr
    return np.ascontiguousarray(
        a.reshape(8, 128, w).transpose(1, 0, 2).reshape(128, 8 * w))


def _scale_of(a, target=192.0):
    m = float(np.abs(a).max())
    if m == 0.0:
        return 1.0
    return float(2.0 ** np.floor(np.log2(target / m)))


def _q8x2(a, s):
    """fp8 hi/lo split of a*s, concatenated along axis 1."""
    hi = (a * s).astype(NP_F8)
    lo = (a * s - hi.astype(np.float32)).astype(NP_F8)
    return np.concatenate([hi, lo], axis=1)


def _host_prep(x, context, x_mask, context_mask, Wq, bq, Wk, bk, Wv, bv,
               Wo, bo):
    f32 = np.float32
    x = np.asarray(x, f32)
    context = np.asarray(context, f32)
    xm = np.asarray(x_mask).astype(f32)
    cm = np.asarray(context_mask).astype(f32)
    Wq, bq, Wk, bk = (np.asarray(a, f32) for a in (Wq, bq, Wk, bk))
    Wv, bv, Wo, bo = (np.asarray(a, f32) for a in (Wv, bv, Wo, bo))

    len_q = xm.sum(axis=(1, 2))
    len_k = cm.sum(axis=(1, 2))
    QKs = _trim_lens(x_mask, context_mask)
    plan = _plan(QKs)

    p = np.arange(128)
    theta = (ROPE_GAMMA /
             (ROTARY_BASE ** (np.arange(0, 32, dtype=f32) / 32))).astype(f32)
    th_p = theta[p % 32]
    sgn = np.where((p % 64) < 32, 1.0, -1.0).astype(f32)[:, None]
    permm = np.zeros((128, 128), f32)
    permm[p, p ^ 32] = 1.0

    # next free pair index per batch
    nxt = [0] * B
    shared = {
        "onesr": np.ones((1, 128), NP_MDT),
        "perm": permm.astype(NP_MDT),
    }
    core_maps, core_sigs, core_segs = [], [], []
    ctxT_c = [np.ascontiguousarray(context[b].T) for b in range(B)]
    for c in range(N_CORES):
        m = dict(shared)
        sig = []
        segs = []
        for si, (b, npair) in enumerate(plan[c]):
            Q, K = QKs[b]
            pair0 = nxt[b]
            nxt[b] += npair
            pairs = list(range(pair0, pair0 + npair))
            segs.append((b, pairs, Q, K))
            sig.append((Q, K, npair))
            nlt = len(_ltiles(K))
            cols = np.concatenate(
                [np.arange(pr * 128, (pr + 1) * 128) for pr in pairs])

            m[f"xs{si}"] = _pack8(x[b][:, :Q], Q).astype(NP_MDT)
            m[f"ctx{si}"] = _pack8(ctxT_c[b][:, :K], K).astype(NP_MDT)
            m[f"wq{si}"] = np.concatenate(
                [_pack8(Wq[:, pr * 128:(pr + 1) * 128], 128)
                 for pr in pairs], axis=1).astype(NP_MDT)
            m[f"wk{si}"] = np.concatenate(
                [_pack8(Wk[:, pr * 128:(pr + 1) * 128], 128)
                 for pr in pairs], axis=1).astype(NP_MDT)
            wv = Wv[:, cols]  # [1024, 128*np]
            m[f"wv{si}"] = np.ascontiguousarray(
                wv.reshape(8, 128, 128 * npair).transpose(1, 0, 2)
                .reshape(128, 8 * 128 * npair)).astype(NP_MDT)
            wo = Wo[cols, :]  # [128*np, 1024]
            m[f"wo{si}"] = np.ascontiguousarray(
                wo.reshape(npair, 128, 1024).transpose(1, 0, 2)
                .reshape(128, 1024 * npair)).astype(NP_MDT)

            pos_q = np.arange(Q, dtype=f32) / len_q[b]
            frq = pos_q[None, :] * th_p[:, None]
            pos_k = np.arange(K, dtype=f32) / len_k[b]
            frk = pos_k[None, :] * th_p[:, None]
            m[f"tg{si}"] = np.concatenate(
                [np.cos(frq), np.sin(frq) * sgn,
                 np.cos(frk), np.sin(frk) * sgn], axis=1).astype(NP_MDT)

            cst = np.zeros((128, 2 * npair + nlt), f32)
            for a, pr in enumerate(pairs):
                cst[:, a] = bq[pr * 128:(pr + 1) * 128]
                cst[:, npair + a] = bk[pr * 128:(pr + 1) * 128]
            for lt, (l0, lsz) in enumerate(_ltiles(K)):
                cst[0:lsz, 2 * npair + lt] = (
                    (cm[b, 0, l0:l0 + lsz] - 1.0) * (-MASK_BIAS))
            m[f"cst{si}"] = cst
            m[f"bvr{si}"] = bv[cols].reshape(1, -1).astype(NP_MDT)
        core_maps.append(m)
        core_sigs.append(tuple(sig))
        core_segs.append(segs)
    assert all(n == 8 for n in nxt), nxt
    return core_maps, core_sigs, core_segs


def kernel(x, context, x_mask, context_mask, Wq, bq, Wk, bk, Wv, bv, Wo, bo):
    import jax

    bass2jax.install_neuronx_cc_hook()
    core_maps, core_sigs, core_segs = _host_prep(
        x, context, x_mask, context_mask, Wq, bq, Wk, bk, Wv, bv, Wo, bo)

    global LAST_CORE_NCS
    LAST_CORE_NCS = []
    devices = jax.devices()[:N_CORES]
    futs = []
    for c in range(N_CORES):
        nc = _get_program(core_sigs[c])
        LAST_CORE_NCS.append(nc)
        jfn, in_names, out_names, zero_outs = _jit_for(nc)
        args = [jax.device_put(np.asarray(core_maps[c][nm]), devices[c])
                for nm in in_names]
        args += [jax.device_put(z, devices[c]) for z in zero_outs]
        futs.append((jfn(*args), out_names))

    out = np.zeros((B, DM, T), np.float32)
    xm = np.asarray(x_mask).astype(np.float32)
    for c in range(N_CORES):
        res, out_names = futs[c]
        ys = {nm: np.asarray(res[i]) for i, nm in enumerate(out_names)}
        for si, (b, pairs, Q, K) in enumerate(core_segs[c]):
            y = ys[f"y{si}"].astype(np.float32)  # [128, 8*Q] bf16 partials
            out[b][:, :Q] += y.reshape(128, 8, Q).transpose(1, 0, 2).reshape(
                1024, Q)
    bo = np.asarray(bo, np.float32)
    for b in range(B):
        out[b] += bo[:, None]
        out[b] *= xm[b, 0][None, :]
    return out

